# revision 74
# baseline (speedup 1.0000x reference)
"""Trainium2 Bass kernel v2 for the dense transformer block.

Sharding: 8-way SPMD, one (batch, half-sequence) shard of Tq=1024 query tokens
per core; K/V recomputed for the full batch (rows host-rolled so local queries
come first). No collectives.

v2 changes vs v1:
- Host-side folds: g1 into wq/wk rows (bq/bk = be1@wq carried as per-channel
  output biases), g2/be2 into w1/b1. LN kernels emit only u=(x-mu)*rstd.
- Phase 1 computes row stats lean (no Newton; ACT Rsqrt), normalizes on ACT
  (scale/bias per-partition), transposes fp16, and also transposes the raw
  rows of the query half once for the residual (kills v1 phase-5 reload).
- S matmuls (K=64) run as row-tiled head pairs: lhsT/rhs at partitions 0-63
  and 64-127 issue to disjoint PE row groups and execute concurrently (~2x).
- attn-proj also row-tiled: O^T stored as [128, Tq] fp16 head-pair tiles
  (odd head partition-shifted 0:64 -> 64:128 via tiny SBUF->SBUF DMA).
- ES/V are fp8 (SBUF relief; matmul rate unchanged), Q/K/O fp16.
- fc1 stays f32r, fc2 fp16 (DoubleRow fp8 measured no faster than f32r on
  this stack, so no fp8 GEMMs).
"""

import sys

import numpy as np

if "/opt/trn_rl_repo" not in sys.path:
    sys.path.insert(0, "/opt/trn_rl_repo")

CFG_FULL = dict(
    Tq=1024, Tkv=2048, C=1024, H=16, D=64, HID=4096, NCLS=1000, EPS=1e-5,
    B=4, N=2048,
)


def _chunks(total, size):
    out = []
    s = 0
    while s < total:
        c = min(size, total - s)
        out.append((s, c))
        s += c
    return out


def emit_block(tc, out_ap, ins, cfg):
    """Emit the full transformer-block program for one core's shard."""
    import concourse.mybir as mybir
    from concourse.masks import make_identity

    nc = tc.nc
    f32 = mybir.dt.float32
    f16 = mybir.dt.float16
    f8 = mybir.dt.float8e4
    f32r = mybir.dt.float32r
    AF = mybir.ActivationFunctionType
    OP = mybir.AluOpType
    AX = mybir.AxisListType

    Tq, Tkv, C, H, D = cfg["Tq"], cfg["Tkv"], cfg["C"], cfg["H"], cfg["D"]
    HID, NCLS, EPS = cfg["HID"], cfg["NCLS"], cfg["EPS"]
    P = 128
    CT = C // P           # 8 channel tiles
    KT = Tkv // P         # 16 key tiles
    HT = HID // P         # 32 hidden tiles
    HPAIR = H // 2        # 8 head pairs
    DA = D + 1
    SCALE = C ** -0.5
    gelu_func = AF.Tanh if cfg.get("sim_gelu_tanh") else AF.Gelu

    assert H * D == C and D == 64 and C % P == 0 and Tkv % P == 0

    def pool(name, bufs=1, space="SBUF"):
        return tc.tile_pool(name=name, bufs=bufs, space=space)

    # ---------------- constants & params ----------------
    const_cm = pool("const")
    const_pool = const_cm.__enter__()

    ident32 = const_pool.tile([P, P], f32)
    make_identity(nc, ident32)
    identr = const_pool.tile([P, P], f32r)
    nc.vector.tensor_copy(identr, ident32)
    ident16 = const_pool.tile([P, P], f16)
    nc.vector.tensor_copy(ident16, ident32)
    ident8 = const_pool.tile([P, P], f8)
    nc.vector.tensor_copy(ident8, ident32)
    ones16 = const_pool.tile([P, P], f16)
    nc.vector.memset(ones16, 1.0)
    ones_r = const_pool.tile([P, P], f32r)
    nc.vector.memset(ones_r.bitcast(f32), 1.0)
    eps_ap = const_pool.tile([P, 1], f32)
    nc.vector.memset(eps_ap, EPS)

    pp = {}
    with pool("ppps", bufs=2, space="PSUM") as psum_misc:
        def load_pp(vec_ap, n, key):
            nt = n // P
            ld = const_pool.tile([nt, P], f32, tag="pp_ld")
            nc.sync.dma_start(ld, vec_ap.rearrange("(a p) -> a p", p=P))
            ps = psum_misc.tile([P, nt], f32, tag="pp_ps")
            nc.tensor.matmul(ps, ld, ident32[:nt, :nt], is_transpose=True)
            dst = const_pool.tile([P, nt], f32, tag=f"pp_{key}")
            nc.vector.tensor_copy(dst, ps)
            pp[key] = dst

        for key in ["bq", "bk", "b_ap", "b2"]:
            load_pp(ins[key], C, key)
        load_pp(ins["b1"], HID, "b1")
    bout_sb = const_pool.tile([1, NCLS], f32r)
    nc.sync.dma_start(bout_sb, ins["b_out"][None, :].bitcast(f32r))

    # ---------------- long-lived arena ----------------
    arena_cm = pool("arena")
    arena = arena_cm.__enter__()

    def atile(slot, shape, dtype, name):
        return arena.tile(shape, dtype, tag=slot, name=name)

    # =========== Phase 1: rows -> LN1 stats -> xn^T (f16) + xq^T ===========
    # Single arena tiles [P, CT, T]; per-ct views keep downstream code shape.
    # Vp ones-columns preset here so the memsets run during the initial
    # DMA wait instead of stalling DVE at phase-3 start.
    Vp = [atile(f"v{i}", [P, 2, H, DA], f8, f"V{i}") for i in range(KT // 2)]
    for _vp in Vp:
        nc.vector.memset(_vp[:, :, :, D:DA], 1.0)
    xq_all = atile("xq_all", [P, CT, Tq], f16, "xq_all")
    xn_all = atile("xn_all", [P, CT, Tkv], f8, "xn_all")
    xq_tiles = [xq_all[:, ct, :] for ct in range(CT)]
    xn_tiles = [xn_all[:, ct, :] for ct in range(CT)]

    # wq/wk/wv DMA'd directly as fp8 (host pre-scaled/converted)
    wqk_cm = pool("wqk")
    wqk_pool = wqk_cm.__enter__()

    def stage_weight(w_ap, key, dt, wpool=None):
        w = (wpool or wqk_pool).tile([P, CT, C], dt, tag=f"w_{key}")
        nc.sync.dma_start(w, w_ap.rearrange("(ct p) m -> p ct m", p=P))
        return w

    QT = [atile(f"q{i}", [P, Tq], f16, f"QT{i}") for i in range(HPAIR)]
    KTt = [atile(f"x1_{i}", [P, Tkv], f16, f"KT{i}") for i in range(HPAIR)]

    with pool("ld1", bufs=2) as load_pool, \
         pool("lnst", bufs=4) as stat_pool, \
         pool("lnsq", bufs=1) as sq_pool, \
         pool("vld", bufs=3) as vload, \
         pool("vt", bufs=3) as vt_pool, \
         pool("qkps", bufs=2, space="PSUM") as qk_psum, \
         pool("trps1", bufs=2, space="PSUM") as tpsum, \
         pool("vps", bufs=2, space="PSUM") as v_psum:

        def emit_qk_chunk(w8, dst, bias, toff):
            """One 512-token chunk of Q- or K-gen for all 8 output tiles.

            fp8 DoubleRow: channel-tile pairs fused per matmul."""
            for m in range(CT):
                ps = qk_psum.tile([P, 512], f32, tag="qk_ps")
                for c in range(CT // 2):
                    nc.tensor.matmul(
                        ps, w8[:, 2 * c:2 * c + 2, m * P:(m + 1) * P],
                        xn_all[:, 2 * c:2 * c + 2, toff:toff + 512],
                        start=(c == 0), stop=(c == CT // 2 - 1),
                        perf_mode=mybir.MatmulPerfMode.DoubleRow)
                nc.vector.tensor_scalar_add(
                    dst[m][:, toff:toff + 512], ps, bias[:, m:m + 1])

        def emit_p1_row(tg):
            r = load_pool.tile([P, C], f16, tag="rowload")
            nc.sync.dma_start(r, ins["xs"][tg * P:(tg + 1) * P, :])
            s1 = stat_pool.tile([P, 1], f32, tag="r_s1")
            nc.vector.reduce_sum(s1, r, axis=AX.X)
            sqt = sq_pool.tile([P, C], f16, tag="r_sq")
            s2 = stat_pool.tile([P, 1], f32, tag="r_s2")
            nc.vector.scalar_tensor_tensor(
                sqt, r, 1.0, r, OP.bypass, OP.mult, accum_out=s2)
            t = stat_pool.tile([P, 1], f32, tag="r_t")
            nc.vector.tensor_tensor(t, s1, s1, OP.mult)
            varp = stat_pool.tile([P, 1], f32, tag="r_varp")
            nc.vector.scalar_tensor_tensor(
                varp, t, -1.0 / C, s2, OP.mult, OP.add)
            std = stat_pool.tile([P, 1], f32, tag="r_std")
            nc.scalar.activation(std, varp, AF.Sqrt,
                                 bias=eps_ap, scale=1.0 / C)
            rstd = stat_pool.tile([P, 1], f32, tag="r_rstd")
            nc.vector.reciprocal(rstd, std)
            r16 = load_pool.tile([P, C], f16, tag="row16")
            if cfg.get("ln1_act_norm"):
                nb = stat_pool.tile([P, 1], f32, tag="r_nb")
                nc.vector.scalar_tensor_tensor(
                    nb, s1, -1.0 / C, rstd, OP.mult, OP.mult)
                nc.scalar.activation(r16, r, AF.Lrelu, bias=nb, scale=rstd,
                                     alpha=1.0)
            else:
                negmu = stat_pool.tile([P, 1], f32, tag="r_nb")
                nc.vector.tensor_scalar_mul(negmu, s1, -1.0 / C)
                nc.vector.tensor_scalar(r16, r, negmu, rstd, OP.add, OP.mult)
            # transpose normalized rows (fp16) -> xn^T fp8 (copy converts)
            ps = tpsum.tile([P, CT, P], f16, tag="tr16")
            for j in range(CT):
                nc.tensor.matmul(
                    ps[:, j, :], r16[:, j * P:(j + 1) * P],
                    ident16, is_transpose=True,
                    start=(j == 0), stop=(j == CT - 1))
            nc.vector.tensor_copy(xn_all[:, :, tg * P:(tg + 1) * P], ps)
            # transpose raw rows (query half) -> xq^T fp16 (rows already f16)
            if tg < Tq // P:
                ps = tpsum.tile([P, CT, P], f16, tag="tr16")
                for j in range(CT):
                    nc.tensor.matmul(
                        ps[:, j, :], r[:, j * P:(j + 1) * P],
                        ident16, is_transpose=True,
                        start=(j == 0), stop=(j == CT - 1))
                nc.vector.tensor_copy(xq_all[:, :, tg * P:(tg + 1) * P], ps)

        def emit_vt(kt):
            # f16 transpose (fp8 PE transpose needs stride-2 out on HW);
            # the PSUM->SBUF copy converts to fp8 for the DoubleRow vmm
            rows = vload.tile([P, C], f16, tag="vrow")
            nc.sync.dma_start(rows, ins["vals"][kt * P:(kt + 1) * P, :])
            vT8 = vt_pool.tile([P, CT, P], f8, tag="vT8")
            ps = tpsum.tile([P, CT, P], f16, tag="tr16")
            for j in range(CT):
                nc.tensor.matmul(
                    ps[:, j, :], rows[:, j * P:(j + 1) * P],
                    ident16, is_transpose=True,
                    start=(j == 0), stop=(j == CT - 1))
            nc.vector.tensor_copy(vT8, ps)
            return vT8

        def emit_vmm(kt, vT8):
            for i, (noff, nsz) in enumerate(_chunks(C, 512)):
                vp = v_psum.tile([P, 512], f32, tag="v_ps")
                for c in range(CT // 2):
                    nc.tensor.matmul(
                        vp, vT8[:, 2 * c:2 * c + 2, :],
                        wv8[:, 2 * c:2 * c + 2, noff:noff + nsz],
                        start=(c == 0), stop=(c == CT // 2 - 1),
                        perf_mode=mybir.MatmulPerfMode.DoubleRow)
                nc.vector.tensor_copy(
                    Vp[kt // 2][:, kt % 2, 8 * i:8 * i + 8, 0:D],
                    vp.rearrange("p (h d) -> p h d", d=D))

        def emit_v_pair(k0):
            a, b = emit_vt(k0), emit_vt(k0 + 1)
            emit_vmm(k0, a)
            emit_vmm(k0 + 1, b)

        # interleave at 512-token granularity: each QK chunk needs only 4
        # LN'd row tiles, so PE matmuls start as soon as rows 0-3 are done.
        # V work (LN-independent) fills the remaining PE gaps, keeping the
        # tensor engine continuously busy (p-state ramp). Weight DMAs are
        # issued AFTER the first row loads so rows aren't queued behind 6MB
        # of weights (single DMA queue = issue order matters).
        for tg in range(2):
            emit_p1_row(tg)
        wv8 = stage_weight(ins["wv"], "wv", f8)
        wq8 = stage_weight(ins["wq"], "wq", f8)
        for tg in range(2, 4):
            emit_p1_row(tg)
        wk8 = stage_weight(ins["wk"], "wk", f8)
        emit_v_pair(0)
        emit_qk_chunk(wq8, QT, pp["bq"], 0)
        for tg in range(4, 8):
            emit_p1_row(tg)
        emit_v_pair(2)
        emit_qk_chunk(wq8, QT, pp["bq"], 512)
        emit_qk_chunk(wk8, KTt, pp["bk"], 0)
        for tg in range(8, 12):
            emit_p1_row(tg)
        emit_v_pair(4)
        emit_qk_chunk(wk8, KTt, pp["bk"], 512)
        for tg in range(12, KT):
            emit_p1_row(tg)
        emit_v_pair(6)
        emit_qk_chunk(wk8, KTt, pp["bk"], 1024)
        emit_v_pair(8)
        emit_qk_chunk(wk8, KTt, pp["bk"], 1536)
        for k0 in range(10, KT, 2):
            emit_v_pair(k0)

    wqk_cm.__exit__(None, None, None)

    if cfg.get("stop_after") in (1, 2, 3):
        arena_cm.__exit__(None, None, None)
        const_cm.__exit__(None, None, None)
        return

    # ====== Phase 4+5: attention (S^T row-tiled pairs; fp8 ES/V DoubleRow)
    # ====== with attn-proj + residual interleaved into the unit loop
    OT = [atile(f"o{i}", [P, Tq], f16, f"OT{i}") for i in range(HPAIR)]
    x1 = [atile(f"x1_{ct}", [P, Tq], f16, f"x1_{ct}") for ct in range(CT)]
    # w_ap DMA'd directly as f16 so attn-proj can start immediately
    # w1 chunk-0 prefetched during attention so fc1 starts immediately
    # (opened before wapp: pools close LIFO, w1f outlives wapp)
    W1CH = 512
    w1f_cm = pool("w1f")
    w1f_pool = w1f_cm.__enter__()
    w1_first = w1f_pool.tile([P, CT, W1CH], f16, tag="w1cb0")
    nc.gpsimd.dma_start(
        w1_first,
        ins["w1"].rearrange("(ct p) m -> p ct m", p=P)[:, :, 0:W1CH])
    wap_cm = pool("wapp")
    wap_pool = wap_cm.__enter__()
    wap16 = wap_pool.tile([P, CT, C], f16, tag="wap16")
    nc.sync.dma_start(wap16, ins["w_ap"].rearrange("(ct p) m -> p ct m", p=P))

    # Schraudolph fast-exp on DVE, writing the fp8e4m3 bit pattern directly
    # as int8 (single DVE pass); ACT takes the rest with exact Exp.
    EXPA8 = float((2 ** 3) * SCALE / np.log(2.0))
    EXPB8 = float((2 ** 3) * (7.0 - 0.0579))
    # per-kt exp engine assignment: A=ACT exact Exp, D=DVE int8-schraudolph,
    # G=gpsimd int8-schraudolph
    exp_pat = cfg.get("exp_pat", "ADADADADADADADAA")
    i8 = mybir.dt.int8

    with pool("es", bufs=2) as es_pool, \
         pool("attsmall", bufs=cfg.get("attsmall_bufs", 1)) as small_pool, \
         pool("apsb", bufs=1) as ap_sb, \
         pool("sps", bufs=cfg.get("sps_bufs", 2), space="PSUM") as s_psum, \
         pool("ops", bufs=cfg.get(
             "ops_bufs", 2 if cfg.get("pb_norm") else 1),
             space="PSUM") as o_psum, \
         pool("apps", bufs=1, space="PSUM") as ap_psum, \
         pool("bcps", bufs=1, space="PSUM") as bc_psum:

        def emit_s_exp(hp, qoff, qsz):
            es = es_pool.tile([P, 2, KT, qsz], f8, tag="es")
            for kt in range(KT):
                sp = s_psum.tile([P, 2, qsz], f32, tag="s_ps")
                nc.tensor.matmul(
                    sp[:, 0, :], KTt[hp][0:64, kt * P:(kt + 1) * P],
                    QT[hp][0:64, qoff:qoff + qsz],
                    start=True, stop=True)
                nc.tensor.matmul(
                    sp[:, 1, :], KTt[hp][64:128, kt * P:(kt + 1) * P],
                    QT[hp][64:128, qoff:qoff + qsz],
                    start=True, stop=True)
                eng = exp_pat[kt]
                if eng == "D":
                    nc.vector.tensor_scalar(
                        es[:, :, kt, :].bitcast(i8), sp, EXPA8, EXPB8,
                        OP.mult, OP.add)
                elif eng == "G":
                    nc.gpsimd.tensor_scalar(
                        es[:, :, kt, :].bitcast(i8), sp, EXPA8, EXPB8,
                        OP.mult, OP.add)
                else:
                    nc.scalar.activation(es[:, :, kt, :], sp, AF.Exp,
                                         bias=0.0, scale=SCALE)
            return es

        def emit_av(hp, qoff, qsz, es):
            for j in (0, 1):
                h = 2 * hp + j
                op = o_psum.tile([DA, qsz], f32, tag="o_ps")
                for t in range(KT // 2):
                    # fp8 DoubleRow: both k-tiles of the pair in one matmul
                    # (lhsT [P,2,DA] -> out [DA,..], rhs [P,2,qsz])
                    nc.tensor.matmul(
                        op, Vp[t][:, :, h, :], es[:, j, 2 * t:2 * t + 2, :],
                        start=(t == 0), stop=(t == KT // 2 - 1),
                        perf_mode=mybir.MatmulPerfMode.DoubleRow)
                bc_sb = small_pool.tile([D, qsz], f32, tag="bc_sb")
                if cfg.get("pb_norm"):
                    # broadcast 1/den from partition 64 to partitions 0-63
                    # on the (otherwise idle) gpsimd engine
                    rec = small_pool.tile([DA, qsz], f32, tag="rec")
                    nc.vector.reciprocal(rec[D:DA, :], op[D:DA, :])
                    nc.gpsimd.partition_broadcast(
                        bc_sb, rec[D:DA, :], channels=D)
                else:
                    rec = small_pool.tile([DA, qsz], f32r, tag="rec")
                    with nc.allow_low_precision(reason="recip feeds matmul"):
                        nc.vector.reciprocal(rec[D:DA, :], op[D:DA, :])
                    bc = bc_psum.tile([D, qsz], f32, tag="bc")
                    nc.tensor.matmul(bc, ones_r[D:D + 1, 0:D],
                                     rec[D:DA, :], start=True, stop=True)
                    nc.scalar.copy(bc_sb, bc)
                if j == 0:
                    nc.vector.tensor_tensor(
                        OT[hp][0:64, qoff:qoff + qsz], op[0:D, :],
                        bc_sb, OP.mult)
                else:
                    otmp = small_pool.tile([D, qsz], f16, tag="otmp")
                    nc.vector.tensor_tensor(
                        otmp, op[0:D, :], bc_sb, OP.mult)
                    nc.sync.dma_start(
                        OT[hp][64:128, qoff:qoff + qsz], otmp)

        def emit_ap_ct(toff, tsz, ct):
            """attn-proj + residual for one (token-chunk, channel-tile)."""
            psA = ap_psum.tile([P, tsz], f32, tag="ap_psA")
            psB = ap_psum.tile([P, tsz], f32, tag="ap_psB")
            for i in range(HPAIR):
                nc.tensor.matmul(
                    psA, wap16[0:64, i, ct * P:(ct + 1) * P],
                    OT[i][0:64, toff:toff + tsz],
                    start=(i == 0), stop=(i == HPAIR - 1))
                nc.tensor.matmul(
                    psB, wap16[64:128, i, ct * P:(ct + 1) * P],
                    OT[i][64:128, toff:toff + tsz],
                    start=(i == 0), stop=(i == HPAIR - 1))
            tA = ap_sb.tile([P, tsz], f32, tag="ap_tA")
            nc.vector.scalar_tensor_tensor(
                tA, psA, pp["b_ap"][:, ct:ct + 1],
                xq_tiles[ct][:, toff:toff + tsz], OP.add, OP.add)
            nc.vector.tensor_tensor(
                x1[ct][:, toff:toff + tsz], tA, psB, OP.add)

        # units qoff-outer (all head-pairs of chunk 0 first) so attn-proj
        # chunk 0 interleaves into the second half of the unit loop, filling
        # PE while ACT/DVE work through the exps.
        units = [(hp, qoff, qsz) for (qoff, qsz) in _chunks(Tq, 512)
                 for hp in range(HPAIR)]
        ap0_done = 0
        prev_unit, prev_es = units[0], emit_s_exp(*units[0])
        for ui, unit in enumerate(units[1:], start=1):
            cur_es = emit_s_exp(*unit)
            emit_av(*prev_unit, prev_es)
            prev_unit, prev_es = unit, cur_es
            if ui >= 9 and ap0_done < CT:
                emit_ap_ct(0, 512, ap0_done)
                ap0_done += 1
        emit_av(*prev_unit, prev_es)
        for ct in range(ap0_done, CT):
            emit_ap_ct(0, 512, ct)
        for ct in range(CT):
            emit_ap_ct(512, 512, ct)

    wap_cm.__exit__(None, None, None)

    if cfg.get("stop_after") in (4, 5):
        w1f_cm.__exit__(None, None, None)
        arena_cm.__exit__(None, None, None)
        const_cm.__exit__(None, None, None)
        return

    # ============ Phase 6+7: LN2 (u only), fc1 + gelu -> h^T fp16 ============
    with pool("ln2ps", bufs=2, space="PSUM") as ln_spool, \
         pool("ln2stat", bufs=1) as ln_stat, \
         pool("w1", bufs=2) as w1_pool, \
         pool("f1ps", bufs=4, space="PSUM") as f1_psum:
        xn2_all = atile("xn_all", [P, CT, Tq], f16, "xn2_all")
        xn2 = [xn2_all[:, ct, :] for ct in range(CT)]
        for (toff, tsz) in _chunks(Tq, 512):
            s1 = ln_spool.tile([P, tsz], f32, tag="ln_s1")
            s2 = ln_spool.tile([P, tsz], f32, tag="ln_s2")
            for ct in range(CT):
                xc = x1[ct][:, toff:toff + tsz]
                nc.tensor.matmul(s1, ones16, xc,
                                 start=(ct == 0), stop=(ct == CT - 1))
                sq = ln_stat.tile([P, tsz], f16, tag="ln_sq")
                nc.vector.tensor_tensor(sq, xc, xc, OP.mult)
                nc.tensor.matmul(s2, ones16, sq,
                                 start=(ct == 0), stop=(ct == CT - 1))
            mu = ln_stat.tile([P, tsz], f32, tag="ln_mu")
            nc.vector.tensor_scalar_mul(mu, s1, 1.0 / C)
            m2 = ln_stat.tile([P, tsz], f32, tag="ln_m2")
            nc.vector.tensor_scalar_mul(m2, s2, 1.0 / C)
            musq = ln_stat.tile([P, tsz], f32, tag="ln_musq")
            nc.vector.tensor_tensor(musq, mu, mu, OP.mult)
            var = ln_stat.tile([P, tsz], f32, tag="ln_var")
            nc.vector.tensor_tensor(var, m2, musq, OP.subtract)
            std = ln_stat.tile([P, tsz], f32, tag="ln_std")
            nc.scalar.activation(std, var, AF.Sqrt,
                                 bias=eps_ap, scale=1.0)
            A = ln_stat.tile([P, tsz], f32, tag="ln_A")
            nc.vector.reciprocal(A, std)
            Bt = ln_stat.tile([P, tsz], f32, tag="ln_B")
            nc.vector.scalar_tensor_tensor(
                Bt, mu, -1.0, A, OP.mult, OP.mult)
            for ct in range(CT):
                u = ln_stat.tile([P, tsz], f32, tag="ln_u")
                nc.vector.tensor_tensor(
                    u, x1[ct][:, toff:toff + tsz], A, OP.mult)
                nc.vector.tensor_tensor(
                    xn2[ct][:, toff:toff + tsz], u, Bt, OP.add)

        hT_x = atile("xq_all", [P, 8, Tq], f16, "hT_x")
        hT = ([atile(f"q{i}", [P, Tq], f16, f"hTq{i}") for i in range(8)]
              + [atile(f"v{i}", [P, Tq], f16, f"hTv{i}") for i in range(8)]
              + [atile(f"o{i}", [P, Tq], f16, f"hTo{i}") for i in range(8)]
              + [hT_x[:, i, :] for i in range(8)])
        w1r = ins["w1"].rearrange("(ct p) m -> p ct m", p=P)
        for ci, (moff, msz) in enumerate(_chunks(HID, W1CH)):
            if ci == 0:
                w1_cb = w1_first  # prefetched before the attention block
            else:
                w1_cb = w1_pool.tile([P, CT, W1CH], f16, tag="w1cb")
                nc.gpsimd.dma_start(
                    w1_cb[:, :, :msz], w1r[:, :, moff:moff + msz])
            for mi in range(msz // P):
                m = (moff + mi * P) // P
                for i, (toff, tsz) in enumerate(_chunks(Tq, 512)):
                    ps = f1_psum.tile([P, 512], f32, tag="f1_ps")
                    for ct in range(CT):
                        nc.tensor.matmul(
                            ps, w1_cb[:, ct, mi * P:(mi + 1) * P],
                            xn2[ct][:, toff:toff + tsz],
                            start=(ct == 0), stop=(ct == CT - 1))
                    nc.scalar.activation(
                        hT[m][:, toff:toff + tsz], ps, gelu_func,
                        bias=pp["b1"][:, m:m + 1], scale=1.0)

    w1f_cm.__exit__(None, None, None)

    if cfg.get("stop_after") == 7:
        arena_cm.__exit__(None, None, None)
        const_cm.__exit__(None, None, None)
        return

    # ================= Phase 8: fc2 + residual 2 (in-place x1) =================
    # wout prefetched here so phase 9's PE never waits on its DMA
    wout_cm = pool("wout")
    wout_pool = wout_cm.__enter__()
    wout_sb = wout_pool.tile([P, CT, NCLS], f16, tag="wout")
    nc.sync.dma_start(
        wout_sb, ins["w_out"].rearrange("(ct p) m -> p ct m", p=P))

    with pool("w2h", bufs=2) as w2h_pool, \
         pool("f2ps", bufs=2, space="PSUM") as f2_psum:
        w2r = ins["w2"].rearrange("(ht p) c -> p ht c", p=P)
        for ct in range(CT):
            w2_f16 = w2h_pool.tile([P, HT, P], f16, tag="w2f16")
            # sync queue: w1 chunk loads hold the gpsimd queue until fc1's
            # end, which would stall fc2's first matmuls by ~10us
            nc.sync.dma_start(w2_f16, w2r[:, :, ct * P:(ct + 1) * P])
            ps = f2_psum.tile([P, 2, 512], f32, tag="f2_ps")
            for i, (toff, tsz) in enumerate(_chunks(Tq, 512)):
                for ht in range(HT):
                    nc.tensor.matmul(
                        ps[:, i, :], w2_f16[:, ht, :],
                        hT[ht][:, toff:toff + tsz],
                        start=(ht == 0), stop=(ht == HT - 1))
            nc.vector.scalar_tensor_tensor(
                x1[ct], ps.rearrange("p a b -> p (a b)"),
                pp["b2"][:, ct:ct + 1],
                x1[ct], OP.add, OP.add)

    if cfg.get("stop_after") == 8:
        wout_cm.__exit__(None, None, None)
        arena_cm.__exit__(None, None, None)
        const_cm.__exit__(None, None, None)
        return

    # ================= Phase 9: out proj + softmax =================
    with pool("smax", bufs=3) as sm_pool, \
         pool("smsmall", bufs=6) as sms_pool, \
         pool("outps", bufs=4, space="PSUM") as out_psum:
        for tt in range(Tq // P):
            # bank-aligned [P, 2, 512] psum; logits occupy [:, :, 0:500]
            ps = out_psum.tile([P, 2, 512], f32, tag="out_ps")
            for i, (noff, nsz) in enumerate(_chunks(NCLS, 500)):
                for ct in range(CT):
                    nc.tensor.matmul(
                        ps[:, i, 0:nsz], x1[ct][:, tt * P:(tt + 1) * P],
                        wout_sb[:, ct, noff:noff + nsz],
                        start=(ct == 0), stop=False)
                nc.tensor.matmul(
                    ps[:, i, 0:nsz], ones_r[0:1, :],
                    bout_sb[0:1, noff:noff + nsz],
                    start=False, stop=True)
            mx = sms_pool.tile([P, 2], f32, tag="sm_mx")
            nc.vector.reduce_max(mx, ps[:, :, 0:500], axis=AX.X)
            m = sms_pool.tile([P, 1], f32, tag="sm_m")
            nc.vector.reduce_max(m, mx, axis=AX.X)
            negm = sms_pool.tile([P, 1], f32, tag="sm_negm")
            nc.vector.tensor_scalar_mul(negm, m, -1.0)
            esb = sm_pool.tile([P, NCLS], f32, tag="sm_e")
            s = sms_pool.tile([P, 1], f32, tag="sm_s")
            nc.scalar.activation(
                esb.rearrange("p (a b) -> p a b", a=2), ps[:, :, 0:500],
                AF.Exp, bias=negm, scale=1.0, accum_out=s)
            rec = sms_pool.tile([P, 1], f32, tag="sm_rec")
            nc.vector.reciprocal(rec, s)
            nc.vector.tensor_scalar_mul(esb, esb, rec)
            nc.sync.dma_start(out_ap[tt * P:(tt + 1) * P, :], esb)

    wout_cm.__exit__(None, None, None)
    arena_cm.__exit__(None, None, None)
    const_cm.__exit__(None, None, None)


# ======================= host entry =======================

def _build_nc(cfg, n_bodies=1):
    import concourse.bacc as bacc
    import concourse.mybir as mybir
    import concourse.tile as tile

    Tq, Tkv, C = cfg["Tq"], cfg["Tkv"], cfg["C"]
    HID, NCLS = cfg["HID"], cfg["NCLS"]
    nc = bacc.Bacc("TRN2", target_bir_lowering=False, debug=False)
    # inputs packed into one dram tensor per dtype: cuts host-side PJRT
    # dispatch cost (per-arg overhead) ~6x vs 15 separate tensors
    layout = _pack_layout(cfg)
    packs = {}
    for pk, (mdt, entries) in layout.items():
        total = sum(int(np.prod(shape)) for _, shape in entries)
        packs[pk] = nc.dram_tensor(pk, [total], getattr(mybir.dt, mdt),
                                   kind="ExternalInput").ap()
    ins = {}
    for pk, (_mdt, entries) in layout.items():
        off = 0
        for name, shape in entries:
            size = int(np.prod(shape))
            ap = packs[pk][off:off + size]
            if len(shape) == 2:
                ap = ap.rearrange("(a b) -> a b", b=shape[1])
            ins[name] = ap
            off += size
    out_ap = nc.dram_tensor("out", [Tq, NCLS], mybir.dt.float32,
                            kind="ExternalOutput").ap()
    with tile.TileContext(nc) as tc:
        for _ in range(n_bodies):
            emit_block(tc, out_ap, ins, cfg)
    nc.finalize()
    return nc


def _pack_layout(cfg):
    """Shared (build + host) layout of the 3 packed input tensors."""
    Tq, Tkv, C = cfg["Tq"], cfg["Tkv"], cfg["C"]
    HID, NCLS = cfg["HID"], cfg["NCLS"]
    return {
        "pk16": ("float16", [
            ("xs", (Tkv, C)), ("vals", (Tkv, C)), ("w_ap", (C, C)),
            ("w1", (C, HID)), ("w2", (HID, C)), ("w_out", (C, NCLS)),
        ]),
        "pk8": ("float8e4", [
            ("wq", (C, C)), ("wk", (C, C)), ("wv", (C, C)),
        ]),
        "pk32": ("float32", [
            ("b_ap", (C,)), ("bq", (C,)), ("bk", (C,)),
            ("b1", (HID,)), ("b2", (C,)), ("b_out", (NCLS,)),
        ]),
    }


def make_in_maps(inputs, cfg):
    """Host-side prep: weight folds (g1->wq/wk + bq/bk, g2/be2->w1/b1),
    dtype conversion, per-core row rolls, packing. Returns 8 input dicts."""
    B, N, Tq = cfg["B"], cfg["N"], cfg["Tq"]
    halves = N // Tq
    import concourse.mybir as mybir

    f = lambda k: np.asarray(inputs[k], np.float32)
    g1, be1 = f("g1"), f("be1")
    g2, be2 = f("g2"), f("be2")
    wq, wk, w1 = f("wq"), f("wk"), f("w1")
    h16 = np.float16
    h8 = mybir.dt.np(mybir.dt.float8e4)
    vals_by_name = {
        "wq": (g1[:, None] * wq).astype(h8),
        "wk": (g1[:, None] * wk).astype(h8),
        "bq": (be1 @ wq).astype(np.float32),
        "bk": (be1 @ wk).astype(np.float32),
        "w1": (g2[:, None] * w1).astype(h16),
        "b1": (f("b1") + be2 @ w1).astype(np.float32),
        "wv": f("wv").astype(h8), "w_ap": f("w_ap").astype(h16),
        "b_ap": f("b_ap"),
        "w2": f("w2").astype(h16), "b2": f("b2"),
        "w_out": f("w_out").astype(h16), "b_out": f("b_out"),
    }
    x = f("x").astype(h16)
    value = f("value").astype(h16)
    layout = _pack_layout(cfg)
    tail16 = np.concatenate(
        [vals_by_name[n].ravel() for n, _ in layout["pk16"][1][2:]])
    pk8 = np.concatenate(
        [vals_by_name[n].ravel() for n, _ in layout["pk8"][1]])
    pk32 = np.concatenate(
        [vals_by_name[n].ravel().astype(np.float32)
         for n, _ in layout["pk32"][1]])
    in_maps = []
    for core in range(8):
        b, hf = core // halves, core % halves
        xs = np.roll(x[b], -hf * Tq, axis=0).ravel()
        vals = np.roll(value[b], -hf * Tq, axis=0).ravel()
        in_maps.append({
            "pk16": np.ascontiguousarray(
                np.concatenate([xs, vals, tail16])),
            "pk8": pk8, "pk32": pk32,
        })
    return in_maps


_NC_CACHE = {}


def kernel(**inputs) -> np.ndarray:
    from concourse.bass_utils import run_bass_kernel_spmd

    cfg = CFG_FULL
    B, N = cfg["B"], cfg["N"]
    Tq, NCLS = cfg["Tq"], cfg["NCLS"]
    halves = N // Tq

    if "full" not in _NC_CACHE:
        _NC_CACHE["full"] = _build_nc(cfg)
    nc = _NC_CACHE["full"]

    in_maps = make_in_maps(inputs, cfg)
    res = run_bass_kernel_spmd(nc, in_maps, core_ids=list(range(8)))
    out = np.empty((B, N, NCLS), dtype=np.float32)
    for core in range(8):
        b, hf = core // halves, core % halves
        out[b, hf * Tq:(hf + 1) * Tq, :] = res.results[core]["out"]
    return out

